# revision 8
# baseline (speedup 1.0000x reference)
"""Trainium2 Bass kernel for a 2-layer GRU backbone with done-mask resets.

Reference semantics per step (PyTorch GRU cell, gate order r,z,n):
    gi = x_t @ w_ih.T + b_ih ; gh = h @ w_hh.T + b_hh
    r = sigmoid(gi_r + gh_r); z = sigmoid(gi_z + gh_z)
    n = tanh(gi_n + r * gh_n)
    h' = (1 - z) * n + z * h          (out_t = top-layer h' pre-mask)
    h  = h' * (1 - m_t)               (both layers)
Returns (outputs [S,B,H], h_final [L,B,H]).

Strategy: batch sharded 8 ways (32 rows/core).  Per core the two layer
scans run as independent interleaved streams; the input-to-hidden
projections (which don't depend on the recurrence) are computed as large
chunked matmuls: Gi0 from the (host-pre-transposed) input x, Gi1 from the
layer-0 outputs collected in an SBUF ring that the per-step transposes
already produce in the required [K, M] layout.  All matmuls are bf16 with
fp32 PSUM accumulation; the h' blend runs in transposed [H-part, B-free]
layout so the hidden state never needs re-transposing between steps.

Per-step partition band: step j of a chunk lives on PSUM/SBUF partitions
[32*(j%4), 32*(j%4)+32) because the Gi chunk matmuls produce 128-row
M-tiles covering 4 consecutive steps.
"""

import sys

sys.path.insert(0, "/opt/trn_rl_repo")

import numpy as np
import ml_dtypes

BF = ml_dtypes.bfloat16

S, B, IN, H, L = 512, 256, 256, 512, 2
NCORES = 8
BL = B // NCORES            # 32 batch rows per core
G = 3 * H                   # 1536 gate columns
C = 16                      # timesteps per pipeline chunk
KH = H // 128               # 4 k-chunks for H contraction
KI = IN // 128              # 2 k-chunks for IN contraction


def build(nc, s_steps=S):
    import concourse.bass as bass
    import concourse.mybir as mybir
    import concourse.tile as tile

    dt = mybir.dt
    bf16, f32 = dt.bfloat16, dt.float32
    AF = mybir.ActivationFunctionType
    OP = mybir.AluOpType
    nch = s_steps // C

    # ---- DRAM parameters (per-core shard, host-prepped layouts) ----
    xT = nc.declare_dram_parameter("xT", [IN, s_steps * BL], bf16, isOutput=False)
    wih0T = nc.declare_dram_parameter("wih0T", [IN, G], bf16, isOutput=False)
    wih1T = nc.declare_dram_parameter("wih1T", [H, G], bf16, isOutput=False)
    whh0T = nc.declare_dram_parameter("whh0T", [H, G], bf16, isOutput=False)
    whh1T = nc.declare_dram_parameter("whh1T", [H, G], bf16, isOutput=False)
    # gi bias (added at Gi eviction): [ (b_ih+b_hh)_rz | b_ih_n ]
    gib0 = nc.declare_dram_parameter("gib0", [1, G], bf16, isOutput=False)
    gib1 = nc.declare_dram_parameter("gib1", [1, G], bf16, isOutput=False)
    # per-step psum rank-1 bias for the n region: b_hh_n
    psb0 = nc.declare_dram_parameter("psb0", [1, H], bf16, isOutput=False)
    psb1 = nc.declare_dram_parameter("psb1", [1, H], bf16, isOutput=False)
    h0T = nc.declare_dram_parameter("h0T", [L, H, BL], bf16, isOutput=False)
    m1m = nc.declare_dram_parameter("m1m", [s_steps, BL], bf16, isOutput=False)  # 1 - done_mask
    ident = nc.declare_dram_parameter("ident", [128, 32], bf16, isOutput=False)  # block-repl I32
    ones = nc.declare_dram_parameter("ones", [1, 128], bf16, isOutput=False)
    outT = nc.declare_dram_parameter(
        "outT", [s_steps, 128, KH, BL], f32, isOutput=True
    )
    hfinT = nc.declare_dram_parameter("hfinT", [L, 128, KH, BL], f32, isOutput=True)

    def bcast_ap(row_ap, parts=128):
        """DRAM AP replicating a row across `parts` partitions (step-0 dim)."""
        return bass.AP(
            tensor=row_ap.tensor, offset=row_ap.offset, ap=[[0, parts]] + row_ap.ap
        )

    with tile.TileContext(nc) as tc:
        with (
            tc.tile_pool(name="const", bufs=1) as const,
            tc.tile_pool(name="xc", bufs=2) as xc_pool,
            tc.tile_pool(name="gi0", bufs=2) as gi0_pool,
            tc.tile_pool(name="gi1", bufs=2) as gi1_pool,
            tc.tile_pool(name="ring", bufs=2) as ring_pool,
            tc.tile_pool(name="mask", bufs=2) as mask_pool,
            tc.tile_pool(name="step", bufs=2) as step_pool,
            tc.tile_pool(name="pg0", bufs=1, space=bass.MemorySpace.PSUM) as pg0,
            tc.tile_pool(name="pg1", bufs=1, space=bass.MemorySpace.PSUM) as pg1,
            tc.tile_pool(name="pms", bufs=2, space=bass.MemorySpace.PSUM) as pms,
        ):
            # ---- constants ----
            w_hh, w_ih = [], []
            for l, (whh, wih) in enumerate(((whh0T, wih0T), (whh1T, wih1T))):
                t_hh = const.tile([128, KH, G], bf16, tag=f"whh{l}")
                for k in range(KH):
                    nc.sync.dma_start(
                        out=t_hh[:, k, :], in_=whh[128 * k : 128 * (k + 1), :]
                    )
                w_hh.append(t_hh)
                kin = KI if l == 0 else KH
                t_ih = const.tile([128, kin, G], bf16, tag=f"wih{l}")
                for k in range(kin):
                    nc.sync.dma_start(
                        out=t_ih[:, k, :], in_=wih[128 * k : 128 * (k + 1), :]
                    )
                w_ih.append(t_ih)

            gib_bc, psb_t = [], []
            for l, (g_p, p_p) in enumerate(((gib0, psb0), (gib1, psb1))):
                t = const.tile([128, G], bf16, tag=f"gib{l}")
                nc.sync.dma_start(out=t[:], in_=bcast_ap(g_p[0]))
                gib_bc.append(t)
                p = const.tile([1, H], bf16, tag=f"psb{l}")
                nc.sync.dma_start(out=p[:], in_=p_p[:])
                psb_t.append(p)

            ident_t = const.tile([128, 32], bf16, tag="ident")
            nc.sync.dma_start(out=ident_t[:], in_=ident[:])
            ones_t = const.tile([1, 128], bf16, tag="ones")
            nc.sync.dma_start(out=ones_t[:], in_=ones[:])

            # persistent hidden state, transposed: hT[l][p, k, b] = h[b, 128k+p]
            hT = []
            for l in range(L):
                t = const.tile([128, KH, BL], bf16, tag=f"hT{l}")
                for k in range(KH):
                    nc.sync.dma_start(
                        out=t[:, k, :], in_=h0T[l, 128 * k : 128 * (k + 1), :]
                    )
                hT.append(t)

            def gi_chunk(l, src_lhsT, gi_tile, kin):
                """gi_tile = src.T @ w_ih.T + gi_bias for C steps (4 M-tiles)."""
                for m in range(C // 4):
                    for sub in range(3):
                        ps = pms.tile([128, 512], f32, tag="ms")
                        nsl = slice(512 * sub, 512 * (sub + 1))
                        for k in range(kin):
                            nc.tensor.matmul(
                                ps[:],
                                src_lhsT(k, m),
                                w_ih[l][:, k, nsl],
                                start=(k == 0),
                                stop=(k == kin - 1),
                            )
                        nc.vector.scalar_tensor_tensor(
                            out=gi_tile[:, m, nsl],
                            in0=ps[:],
                            scalar=1.0,
                            in1=gib_bc[l][:, nsl],
                            op0=OP.bypass,
                            op1=OP.add,
                        )

            def step(l, t_glob, j, gi_tile, mask_tile, ring_tile):
                """One scan step for layer l at chunk-local step j."""
                kb = j % 4
                row = slice(32 * kb, 32 * (kb + 1))
                gi = gi_tile[row, j // 4, :]

                # gh matmul into psum (+ rank-1 b_hh_n on the n region)
                pg = pg0 if l == 0 else pg1
                prz = pg.tile([128, 2 * H], f32, tag=f"rz{l}")
                pn = pg.tile([128, H], f32, tag=f"n{l}")
                tp = (0, 32 * kb)
                for s2 in range(2):
                    ssl = slice(512 * s2, 512 * (s2 + 1))
                    for k in range(KH):
                        nc.tensor.matmul(
                            prz[row, ssl],
                            hT[l][:, k, :],
                            w_hh[l][:, k, ssl],
                            start=(k == 0),
                            stop=(k == KH - 1),
                            tile_position=tp,
                        )
                for k in range(KH):
                    nc.tensor.matmul(
                        pn[row, :],
                        hT[l][:, k, :],
                        w_hh[l][:, k, 2 * H :],
                        start=(k == 0),
                        stop=False,
                        tile_position=tp,
                    )
                nc.tensor.matmul(
                    pn[row, :], ones_t[:, :BL], psb_t[l][:],
                    start=False, stop=True, tile_position=tp,
                )

                # r,z = sigmoid(psum_rz + gi_rz)
                rz_s = step_pool.tile([128, 2 * H], bf16, tag=f"rzs{l}")
                nc.vector.scalar_tensor_tensor(
                    out=rz_s[row, :], in0=prz[row, :], scalar=1.0,
                    in1=gi[:, : 2 * H], op0=OP.bypass, op1=OP.add,
                )
                rz_a = step_pool.tile([128, 2 * H], bf16, tag=f"rza{l}")
                nc.scalar.activation(rz_a[row, :], rz_s[row, :], AF.Sigmoid)

                # n = tanh(gi_n + r * psum_n)
                nt = step_pool.tile([128, H], bf16, tag=f"nt{l}")
                nc.vector.scalar_tensor_tensor(
                    out=nt[row, :], in0=pn[row, :], scalar=1.0,
                    in1=rz_a[row, :H], op0=OP.bypass, op1=OP.mult,
                )
                nt2 = step_pool.tile([128, H], bf16, tag=f"nt2{l}")
                nc.vector.tensor_add(nt2[row, :], nt[row, :], gi[:, 2 * H :])
                n_t = step_pool.tile([128, H], bf16, tag=f"na{l}")
                nc.scalar.activation(n_t[row, :], nt2[row, :], AF.Tanh)

                # transpose n and z into [H-part, B-free] layout
                idn = ident_t[row, :]
                tpt = (32 * kb, 0)
                pt_n = pms.tile([128, 128], bf16, tag="ms")
                for i in range(KH):
                    nc.tensor.transpose(
                        pt_n[:, 32 * i : 32 * (i + 1)],
                        n_t[row, 128 * i : 128 * (i + 1)],
                        idn,
                        tile_position=tpt,
                    )
                nT = step_pool.tile([128, KH, BL], bf16, tag=f"nT{l}")
                nc.vector.tensor_copy(
                    nT[:], pt_n[:].rearrange("p (k b) -> p k b", k=KH)
                )
                pt_z = pms.tile([128, 128], bf16, tag="ms")
                for i in range(KH):
                    nc.tensor.transpose(
                        pt_z[:, 32 * i : 32 * (i + 1)],
                        rz_a[row, H + 128 * i : H + 128 * (i + 1)],
                        idn,
                        tile_position=tpt,
                    )
                zT = step_pool.tile([128, KH, BL], bf16, tag=f"zT{l}")
                nc.vector.tensor_copy(
                    zT[:], pt_z[:].rearrange("p (k b) -> p k b", k=KH)
                )

                # h' = n + z*(h - n)   (transposed layout)
                d_t = step_pool.tile([128, KH, BL], bf16, tag=f"d{l}")
                nc.vector.tensor_sub(d_t[:], hT[l][:], nT[:])
                e_t = step_pool.tile([128, KH, BL], bf16, tag=f"e{l}")
                nc.vector.tensor_mul(e_t[:], d_t[:], zT[:])

                mk = mask_tile[:, None, j, :].broadcast_to([128, KH, BL])
                if l == 0:
                    hp = ring_tile[:, :, j, :]  # h' (pre-mask) -> Gi1 lhsT ring
                    nc.vector.tensor_add(hp, e_t[:], nT[:])
                    nc.vector.tensor_mul(hT[0][:], hp, mk)
                    return None
                hp1 = step_pool.tile([128, KH, BL], f32, tag="hp1")
                nc.vector.tensor_add(hp1[:], e_t[:], nT[:])
                nc.sync.dma_start(out=outT[t_glob], in_=hp1[:])
                hb = step_pool.tile([128, KH, BL], bf16, tag="hb1")
                nc.scalar.activation(hb[:], hp1[:], AF.Copy)
                nc.vector.tensor_mul(hT[1][:], hb[:], mk)
                return hp1

            last_hp1 = None
            for c in range(nch):
                # -- load x chunk; compute Gi0 chunk --
                xc = xc_pool.tile([128, KI, C * BL], bf16)
                for k in range(KI):
                    nc.sync.dma_start(
                        out=xc[:, k, :],
                        in_=xT[
                            128 * k : 128 * (k + 1), c * C * BL : (c + 1) * C * BL
                        ],
                    )
                gi0 = gi0_pool.tile([128, C // 4, G], bf16)
                gi_chunk(
                    0, lambda k, m: xc[:, k, 128 * m : 128 * (m + 1)], gi0, KI
                )

                # -- (1 - done_mask) chunk, broadcast across partitions --
                mtile = mask_pool.tile([128, C, BL], bf16)
                nc.sync.dma_start(
                    out=mtile[:], in_=bcast_ap(m1m[c * C : (c + 1) * C, :])
                )

                # -- layer-0 scan over the chunk (fills the ring) --
                ring = ring_pool.tile([128, KH, C, BL], bf16)
                for j in range(C):
                    step(0, c * C + j, j, gi0, mtile, ring)

                # -- Gi1 chunk from the ring --
                gi1 = gi1_pool.tile([128, C // 4, G], bf16)
                gi_chunk(
                    1, lambda k, m: ring[:, k, 4 * m : 4 * (m + 1), :], gi1, KH
                )

                # -- layer-1 scan over the chunk --
                for j in range(C):
                    hp1 = step(1, c * C + j, j, gi1, mtile, None)
                if c == nch - 1:
                    last_hp1 = hp1
                    # final-step layer-0 h' (pre-mask) in f32 for h_final
                    hf0 = step_pool.tile([128, KH, BL], f32, tag="hf0")
                    nc.vector.tensor_copy(hf0[:], ring[:, :, C - 1, :])
                    nc.sync.dma_start(out=hfinT[0], in_=hf0[:])

            nc.sync.dma_start(out=hfinT[1], in_=last_hp1[:])

    return nc


def _prep_core(inputs, core, s_steps=S):
    """Build the per-core input map (host-side layout/dtype prep only)."""
    b0 = core * BL
    x = np.asarray(inputs["x"])[:s_steps, b0 : b0 + BL, :]        # [S, BL, IN]
    hidden = np.asarray(inputs["hidden"])[:, b0 : b0 + BL, :]     # [L, BL, H]
    dm = np.asarray(inputs["done_mask"])[:s_steps, b0 : b0 + BL]  # [S, BL]

    m = {}
    m["xT"] = np.ascontiguousarray(
        x.transpose(2, 0, 1).reshape(IN, s_steps * BL)
    ).astype(BF)
    m["h0T"] = np.ascontiguousarray(hidden.transpose(0, 2, 1)).astype(BF)
    m["m1m"] = (1.0 - dm).astype(BF)
    for l in range(L):
        wi = np.asarray(inputs[f"w_ih{l}"]).astype(np.float32)
        wh = np.asarray(inputs[f"w_hh{l}"]).astype(np.float32)
        bi = np.asarray(inputs[f"b_ih{l}"]).astype(np.float32)
        bh = np.asarray(inputs[f"b_hh{l}"]).astype(np.float32)
        m[f"wih{l}T"] = np.ascontiguousarray(wi.T).astype(BF)
        m[f"whh{l}T"] = np.ascontiguousarray(wh.T).astype(BF)
        gib = np.concatenate([bi[: 2 * H] + bh[: 2 * H], bi[2 * H :]])
        m[f"gib{l}"] = gib.reshape(1, G).astype(BF)
        m[f"psb{l}"] = bh[2 * H :].reshape(1, H).astype(BF)
    eye = np.zeros((128, 32), np.float32)
    for p in range(128):
        eye[p, p % 32] = 1.0
    m["ident"] = eye.astype(BF)
    m["ones"] = np.ones((1, 128), np.float32).astype(BF)
    return m


def _assemble(results, inputs, s_steps=S):
    """Gather per-core outputs back to full [S,B,H] / [L,B,H] fp32."""
    dm = np.asarray(inputs["done_mask"])[:s_steps]
    out = np.empty((s_steps, B, H), np.float32)
    hfin = np.empty((L, B, H), np.float32)
    for core in range(NCORES):
        b0 = core * BL
        oT = np.asarray(results[core]["outT"])    # [S, 128, KH, BL]
        out[:, b0 : b0 + BL, :] = oT.transpose(0, 3, 2, 1).reshape(s_steps, BL, H)
        hT_ = np.asarray(results[core]["hfinT"])  # [L, 128, KH, BL]
        hfin[:, b0 : b0 + BL, :] = hT_.transpose(0, 3, 2, 1).reshape(L, BL, H)
    hfin *= (1.0 - dm[s_steps - 1])[None, :, None]
    return out, hfin


def run(inputs, **spmd_kwargs):
    """Build, execute on 8 cores, and assemble. Returns (out, hfin, results)."""
    import concourse.bacc as bacc
    from concourse.bass_utils import run_bass_kernel_spmd

    nc = bacc.Bacc(None)
    build(nc)
    nc.compile()
    in_maps = [_prep_core(inputs, core) for core in range(NCORES)]
    res = run_bass_kernel_spmd(nc, in_maps, list(range(NCORES)), **spmd_kwargs)
    out, hfin = _assemble(res.results, inputs)
    return out, hfin, res


def kernel(**inputs):
    out, hfin, _ = run(inputs)
    return out, hfin


# revision 9
# speedup vs baseline: 1.5788x; 1.5788x over previous
"""Trainium2 Bass kernel for a 2-layer GRU backbone with done-mask resets.

Reference semantics per step (PyTorch GRU cell, gate order r,z,n):
    gi = x_t @ w_ih.T + b_ih ; gh = h @ w_hh.T + b_hh
    r = sigmoid(gi_r + gh_r); z = sigmoid(gi_z + gh_z)
    n = tanh(gi_n + r * gh_n)
    h' = (1 - z) * n + z * h          (out_t = top-layer h' pre-mask)
    h  = h' * (1 - m_t)               (both layers)
Returns (outputs [S,B,H], h_final [L,B,H]).

Strategy: batch sharded 8 ways (32 rows/core).  Per core the two layer
scans run as independent interleaved streams; the input-to-hidden
projections (which don't depend on the recurrence) are computed as large
chunked matmuls: Gi0 from the (host-pre-transposed) input x, Gi1 from the
layer-0 outputs collected in an SBUF ring that the per-step transposes
already produce in the required [K, M] layout.  All matmuls are bf16 with
fp32 PSUM accumulation; the h' blend runs in transposed [H-part, B-free]
layout so the hidden state never needs re-transposing between steps.

Per-step partition band: step j of a chunk lives on PSUM/SBUF partitions
[32*(j%4), 32*(j%4)+32) because the Gi chunk matmuls produce 128-row
M-tiles covering 4 consecutive steps.
"""

import sys

sys.path.insert(0, "/opt/trn_rl_repo")

import numpy as np
import ml_dtypes

BF = ml_dtypes.bfloat16

S, B, IN, H, L = 512, 256, 256, 512, 2
NCORES = 8
BL = B // NCORES            # 32 batch rows per core
G = 3 * H                   # 1536 gate columns
C = 16                      # timesteps per pipeline chunk
KH = H // 128               # 4 k-chunks for H contraction
KI = IN // 128              # 2 k-chunks for IN contraction


def build(nc, s_steps=S):
    import concourse.bass as bass
    import concourse.mybir as mybir
    import concourse.tile as tile

    dt = mybir.dt
    bf16, f32 = dt.bfloat16, dt.float32
    AF = mybir.ActivationFunctionType
    OP = mybir.AluOpType
    nch = s_steps // C

    # ---- DRAM parameters (per-core shard, host-prepped layouts) ----
    xT = nc.declare_dram_parameter("xT", [IN, s_steps * BL], bf16, isOutput=False)
    wih0T = nc.declare_dram_parameter("wih0T", [IN, G], bf16, isOutput=False)
    wih1T = nc.declare_dram_parameter("wih1T", [H, G], bf16, isOutput=False)
    whh0T = nc.declare_dram_parameter("whh0T", [H, G], bf16, isOutput=False)
    whh1T = nc.declare_dram_parameter("whh1T", [H, G], bf16, isOutput=False)
    # gi bias (added at Gi eviction): [ (b_ih+b_hh)_rz | b_ih_n ]
    gib0 = nc.declare_dram_parameter("gib0", [1, G], bf16, isOutput=False)
    gib1 = nc.declare_dram_parameter("gib1", [1, G], bf16, isOutput=False)
    # per-step psum rank-1 bias for the n region: b_hh_n
    psb0 = nc.declare_dram_parameter("psb0", [1, H], bf16, isOutput=False)
    psb1 = nc.declare_dram_parameter("psb1", [1, H], bf16, isOutput=False)
    h0T = nc.declare_dram_parameter("h0T", [L, H, BL], bf16, isOutput=False)
    m1m = nc.declare_dram_parameter("m1m", [s_steps, BL], bf16, isOutput=False)  # 1 - done_mask
    ident = nc.declare_dram_parameter("ident", [128, 32], bf16, isOutput=False)  # block-repl I32
    ones = nc.declare_dram_parameter("ones", [1, 128], bf16, isOutput=False)
    outT = nc.declare_dram_parameter(
        "outT", [s_steps, 128, KH, BL], f32, isOutput=True
    )
    hfinT = nc.declare_dram_parameter("hfinT", [L, 128, KH, BL], f32, isOutput=True)

    def bcast_ap(row_ap, parts=128):
        """DRAM AP replicating a row across `parts` partitions (step-0 dim)."""
        return bass.AP(
            tensor=row_ap.tensor, offset=row_ap.offset, ap=[[0, parts]] + row_ap.ap
        )

    with tile.TileContext(nc) as tc:
        with (
            tc.tile_pool(name="const", bufs=1) as const,
            tc.tile_pool(name="xc", bufs=2) as xc_pool,
            tc.tile_pool(name="gi0", bufs=2) as gi0_pool,
            tc.tile_pool(name="gi1", bufs=2) as gi1_pool,
            tc.tile_pool(name="ring", bufs=2) as ring_pool,
            tc.tile_pool(name="mask", bufs=2) as mask_pool,
            tc.tile_pool(name="step", bufs=2) as step_pool,
            tc.tile_pool(name="pg0", bufs=1, space=bass.MemorySpace.PSUM) as pg0,
            tc.tile_pool(name="pg1", bufs=1, space=bass.MemorySpace.PSUM) as pg1,
            tc.tile_pool(name="pms", bufs=2, space=bass.MemorySpace.PSUM) as pms,
        ):
            # ---- constants ----
            w_hh, w_ih = [], []
            for l, (whh, wih) in enumerate(((whh0T, wih0T), (whh1T, wih1T))):
                t_hh = const.tile([128, KH, G], bf16, tag=f"whh{l}")
                for k in range(KH):
                    nc.sync.dma_start(
                        out=t_hh[:, k, :], in_=whh[128 * k : 128 * (k + 1), :]
                    )
                w_hh.append(t_hh)
                kin = KI if l == 0 else KH
                t_ih = const.tile([128, kin, G], bf16, tag=f"wih{l}")
                for k in range(kin):
                    nc.sync.dma_start(
                        out=t_ih[:, k, :], in_=wih[128 * k : 128 * (k + 1), :]
                    )
                w_ih.append(t_ih)

            gib_bc, psb_t = [], []
            for l, (g_p, p_p) in enumerate(((gib0, psb0), (gib1, psb1))):
                t = const.tile([128, G], bf16, tag=f"gib{l}")
                nc.sync.dma_start(out=t[:], in_=bcast_ap(g_p[0]))
                gib_bc.append(t)
                p = const.tile([1, H], bf16, tag=f"psb{l}")
                nc.sync.dma_start(out=p[:], in_=p_p[:])
                psb_t.append(p)

            ident_t = const.tile([128, 32], bf16, tag="ident")
            nc.sync.dma_start(out=ident_t[:], in_=ident[:])
            ones_t = const.tile([1, 128], bf16, tag="ones")
            nc.sync.dma_start(out=ones_t[:], in_=ones[:])

            # persistent hidden state, transposed: hT[l][p, k, b] = h[b, 128k+p]
            hT = []
            for l in range(L):
                t = const.tile([128, KH, BL], bf16, tag=f"hT{l}")
                for k in range(KH):
                    nc.sync.dma_start(
                        out=t[:, k, :], in_=h0T[l, 128 * k : 128 * (k + 1), :]
                    )
                hT.append(t)

            def gi_chunk(l, src_lhsT, gi_tile, kin):
                """gi_tile = src.T @ w_ih.T + gi_bias for C steps (4 M-tiles)."""
                for m in range(C // 4):
                    for sub in range(3):
                        ps = pms.tile([128, 512], f32, tag="ms")
                        nsl = slice(512 * sub, 512 * (sub + 1))
                        for k in range(kin):
                            nc.tensor.matmul(
                                ps[:],
                                src_lhsT(k, m),
                                w_ih[l][:, k, nsl],
                                start=(k == 0),
                                stop=(k == kin - 1),
                            )
                        nc.vector.scalar_tensor_tensor(
                            out=gi_tile[:, m, nsl],
                            in0=ps[:],
                            scalar=1.0,
                            in1=gib_bc[l][:, nsl],
                            op0=OP.bypass,
                            op1=OP.add,
                        )

            def step(l, t_glob, j, gi_tile, mask_tile, ring_tile):
                """One scan step for layer l at chunk-local step j."""
                kb = j % 4
                row = slice(32 * kb, 32 * (kb + 1))
                gi = gi_tile[row, j // 4, :]

                # gh matmul into psum (+ rank-1 b_hh_n on the n region)
                pg = pg0 if l == 0 else pg1
                prz = pg.tile([128, 2 * H], f32, tag=f"rz{l}")
                pn = pg.tile([128, H], f32, tag=f"n{l}")
                tp = (0, 32 * kb)
                for s2 in range(2):
                    ssl = slice(512 * s2, 512 * (s2 + 1))
                    for k in range(KH):
                        nc.tensor.matmul(
                            prz[row, ssl],
                            hT[l][:, k, :],
                            w_hh[l][:, k, ssl],
                            start=(k == 0),
                            stop=(k == KH - 1),
                            tile_position=tp,
                        )
                for k in range(KH):
                    nc.tensor.matmul(
                        pn[row, :],
                        hT[l][:, k, :],
                        w_hh[l][:, k, 2 * H :],
                        start=(k == 0),
                        stop=False,
                        tile_position=tp,
                    )
                nc.tensor.matmul(
                    pn[row, :], ones_t[:, :BL], psb_t[l][:],
                    start=False, stop=True, tile_position=tp,
                )

                # r,z = sigmoid(psum_rz + gi_rz)
                rz_s = step_pool.tile([128, 2 * H], bf16, tag=f"rzs{l}")
                nc.vector.scalar_tensor_tensor(
                    out=rz_s[row, :], in0=prz[row, :], scalar=1.0,
                    in1=gi[:, : 2 * H], op0=OP.bypass, op1=OP.add,
                )
                rz_a = step_pool.tile([128, 2 * H], bf16, tag=f"rza{l}")
                nc.scalar.activation(rz_a[row, :], rz_s[row, :], AF.Sigmoid)

                # n = tanh(gi_n + r * psum_n)
                nt = step_pool.tile([128, H], bf16, tag=f"nt{l}")
                nc.vector.scalar_tensor_tensor(
                    out=nt[row, :], in0=pn[row, :], scalar=1.0,
                    in1=rz_a[row, :H], op0=OP.bypass, op1=OP.mult,
                )
                nt2 = step_pool.tile([128, H], bf16, tag=f"nt2{l}")
                nc.vector.tensor_add(nt2[row, :], nt[row, :], gi[:, 2 * H :])
                n_t = step_pool.tile([128, H], bf16, tag=f"na{l}")
                nc.scalar.activation(n_t[row, :], nt2[row, :], AF.Tanh)

                # transpose n and z into [H-part, B-free] layout
                idn = ident_t[row, :]
                tpt = (32 * kb, 0)
                pt_n = pms.tile([128, 128], bf16, tag="ms")
                for i in range(KH):
                    nc.tensor.transpose(
                        pt_n[:, 32 * i : 32 * (i + 1)],
                        n_t[row, 128 * i : 128 * (i + 1)],
                        idn,
                        tile_position=tpt,
                    )
                nT = step_pool.tile([128, KH, BL], bf16, tag=f"nT{l}")
                nc.vector.tensor_copy(
                    nT[:], pt_n[:].rearrange("p (k b) -> p k b", k=KH)
                )
                pt_z = pms.tile([128, 128], bf16, tag="ms")
                for i in range(KH):
                    nc.tensor.transpose(
                        pt_z[:, 32 * i : 32 * (i + 1)],
                        rz_a[row, H + 128 * i : H + 128 * (i + 1)],
                        idn,
                        tile_position=tpt,
                    )
                zT = step_pool.tile([128, KH, BL], bf16, tag=f"zT{l}")
                nc.vector.tensor_copy(
                    zT[:], pt_z[:].rearrange("p (k b) -> p k b", k=KH)
                )

                # h' = n + z*(h - n)   (transposed layout)
                d_t = step_pool.tile([128, KH, BL], bf16, tag=f"d{l}")
                nc.vector.tensor_sub(d_t[:], hT[l][:], nT[:])
                e_t = step_pool.tile([128, KH, BL], bf16, tag=f"e{l}")
                nc.vector.tensor_mul(e_t[:], d_t[:], zT[:])

                mk = mask_tile[:, None, j, :].broadcast_to([128, KH, BL])
                if l == 0:
                    hp = ring_tile[:, :, j, :]  # h' (pre-mask) -> Gi1 lhsT ring
                    nc.vector.tensor_add(hp, e_t[:], nT[:])
                    nc.vector.tensor_mul(hT[0][:], hp, mk)
                    return None
                hp1 = step_pool.tile([128, KH, BL], f32, tag="hp1")
                nc.vector.tensor_add(hp1[:], e_t[:], nT[:])
                nc.sync.dma_start(out=outT[t_glob], in_=hp1[:])
                hb = step_pool.tile([128, KH, BL], bf16, tag="hb1")
                nc.scalar.activation(hb[:], hp1[:], AF.Copy)
                nc.vector.tensor_mul(hT[1][:], hb[:], mk)
                return hp1

            def load_chunk(c):
                """DMA x chunk + mask chunk, compute Gi0 chunk."""
                xc = xc_pool.tile([128, KI, C * BL], bf16)
                for k in range(KI):
                    nc.sync.dma_start(
                        out=xc[:, k, :],
                        in_=xT[
                            128 * k : 128 * (k + 1), c * C * BL : (c + 1) * C * BL
                        ],
                    )
                gi0 = gi0_pool.tile([128, C // 4, G], bf16)
                gi_chunk(
                    0, lambda k, m: xc[:, k, 128 * m : 128 * (m + 1)], gi0, KI
                )
                mtile = mask_pool.tile([128, C, BL], bf16)
                nc.sync.dma_start(
                    out=mtile[:], in_=bcast_ap(m1m[c * C : (c + 1) * C, :])
                )
                return gi0, mtile

            # Layer 1 lags layer 0 by one chunk; their steps are emitted
            # interleaved so the scheduler overlaps the two serial chains
            # (one stream's matmuls fill the other stream's gate-math gaps).
            last_hp1 = None
            gi0, mtile0 = load_chunk(0)
            prev = None  # (gi1, mtile, ring) of the lagging layer-1 chunk
            for c in range(nch + 1):
                ring = None
                if c < nch:
                    ring = ring_pool.tile([128, KH, C, BL], bf16)
                for j in range(C):
                    if c < nch:
                        step(0, c * C + j, j, gi0, mtile0, ring)
                    if prev is not None:
                        hp1 = step(1, (c - 1) * C + j, j, prev[0], prev[1], None)
                if c < nch:
                    # Gi1 for this chunk (consumed by layer 1 next iteration)
                    gi1 = gi1_pool.tile([128, C // 4, G], bf16)
                    gi_chunk(
                        1, lambda k, m: ring[:, k, 4 * m : 4 * (m + 1), :], gi1, KH
                    )
                    if c == nch - 1:
                        # final-step layer-0 h' (pre-mask) in f32 for h_final
                        hf0 = step_pool.tile([128, KH, BL], f32, tag="hf0")
                        nc.vector.tensor_copy(hf0[:], ring[:, :, C - 1, :])
                        nc.sync.dma_start(out=hfinT[0], in_=hf0[:])
                    prev = (gi1, mtile0)
                    if c + 1 < nch:
                        gi0, mtile0 = load_chunk(c + 1)
            last_hp1 = hp1

            nc.sync.dma_start(out=hfinT[1], in_=last_hp1[:])

    return nc


def _prep_core(inputs, core, s_steps=S):
    """Build the per-core input map (host-side layout/dtype prep only)."""
    b0 = core * BL
    x = np.asarray(inputs["x"])[:s_steps, b0 : b0 + BL, :]        # [S, BL, IN]
    hidden = np.asarray(inputs["hidden"])[:, b0 : b0 + BL, :]     # [L, BL, H]
    dm = np.asarray(inputs["done_mask"])[:s_steps, b0 : b0 + BL]  # [S, BL]

    m = {}
    m["xT"] = np.ascontiguousarray(
        x.transpose(2, 0, 1).reshape(IN, s_steps * BL)
    ).astype(BF)
    m["h0T"] = np.ascontiguousarray(hidden.transpose(0, 2, 1)).astype(BF)
    m["m1m"] = (1.0 - dm).astype(BF)
    for l in range(L):
        wi = np.asarray(inputs[f"w_ih{l}"]).astype(np.float32)
        wh = np.asarray(inputs[f"w_hh{l}"]).astype(np.float32)
        bi = np.asarray(inputs[f"b_ih{l}"]).astype(np.float32)
        bh = np.asarray(inputs[f"b_hh{l}"]).astype(np.float32)
        m[f"wih{l}T"] = np.ascontiguousarray(wi.T).astype(BF)
        m[f"whh{l}T"] = np.ascontiguousarray(wh.T).astype(BF)
        gib = np.concatenate([bi[: 2 * H] + bh[: 2 * H], bi[2 * H :]])
        m[f"gib{l}"] = gib.reshape(1, G).astype(BF)
        m[f"psb{l}"] = bh[2 * H :].reshape(1, H).astype(BF)
    eye = np.zeros((128, 32), np.float32)
    for p in range(128):
        eye[p, p % 32] = 1.0
    m["ident"] = eye.astype(BF)
    m["ones"] = np.ones((1, 128), np.float32).astype(BF)
    return m


def _assemble(results, inputs, s_steps=S):
    """Gather per-core outputs back to full [S,B,H] / [L,B,H] fp32."""
    dm = np.asarray(inputs["done_mask"])[:s_steps]
    out = np.empty((s_steps, B, H), np.float32)
    hfin = np.empty((L, B, H), np.float32)
    for core in range(NCORES):
        b0 = core * BL
        oT = np.asarray(results[core]["outT"])    # [S, 128, KH, BL]
        out[:, b0 : b0 + BL, :] = oT.transpose(0, 3, 2, 1).reshape(s_steps, BL, H)
        hT_ = np.asarray(results[core]["hfinT"])  # [L, 128, KH, BL]
        hfin[:, b0 : b0 + BL, :] = hT_.transpose(0, 3, 2, 1).reshape(L, BL, H)
    hfin *= (1.0 - dm[s_steps - 1])[None, :, None]
    return out, hfin


def run(inputs, **spmd_kwargs):
    """Build, execute on 8 cores, and assemble. Returns (out, hfin, results)."""
    import concourse.bacc as bacc
    from concourse.bass_utils import run_bass_kernel_spmd

    nc = bacc.Bacc(None)
    build(nc)
    nc.compile()
    in_maps = [_prep_core(inputs, core) for core in range(NCORES)]
    res = run_bass_kernel_spmd(nc, in_maps, list(range(NCORES)), **spmd_kwargs)
    out, hfin = _assemble(res.results, inputs)
    return out, hfin, res


def kernel(**inputs):
    out, hfin, _ = run(inputs)
    return out, hfin
